# revision 12
# baseline (speedup 1.0000x reference)
"""Trainium2 Bass kernel for the retrieval-KNN module:

    h   = y @ Wy_w.T + Wy_b                      # [B,N,1024]
    dz  = dic_z @ Wz_w.T + Wz_b                  # [K,1024]
    att = softmax(h @ dz.T / sqrt(1024))         # [B,N,K]
    z   = einsum('bnk,k,ke->bne', att, prior, dic_z)

Strategy: data-parallel over B across 8 NeuronCores (8 batches = 2048
tokens per core); dic_z / weights replicated. All matmuls in bf16 (PE
fp32 runs at 1/4 rate), fp32 PSUM accumulation, transposes done by the
DMA transpose xbar on bf16 staged copies.  softmax has no max-subtraction
(logits are O(+-5) for this distribution) and folds the prior in as an
exp() bias: Ep = exp(logits/32 + log(prior)); then
z = (Ep.T @ dic_z) / (Ep.T @ (1/prior)) per token.
"""

import sys

import numpy as np


def _ensure_paths():
    for p in ("/opt/trn_rl_repo",):
        if p not in sys.path:
            sys.path.append(p)


_ensure_paths()

from contextlib import ExitStack  # noqa: E402

import concourse.bacc as bacc  # noqa: E402
import concourse.mybir as mybir  # noqa: E402
import concourse.tile as tile  # noqa: E402
from concourse import bass_utils  # noqa: E402
from concourse.bass import ts  # noqa: E402

F32 = mybir.dt.float32
BF16 = mybir.dt.bfloat16
AF = mybir.ActivationFunctionType

NCORES = 8
# Full problem dims (hardcoded per spec nn_Causal_v_69054484185473)
B, N, EMB = 64, 256, 1024
FULL = dict(T=(B // NCORES) * N, I=1024, O=1024, J=2048, K=4096,
            TC=512, KG=512, EC=512)
SCALE = 1.0 / 32.0  # 1/sqrt(EMB)


def build_bass(T=2048, I=1024, O=1024, J=2048, K=4096, TC=512, KG=512,
               EC=512, dt_mm=BF16, scale=SCALE, num_devices=NCORES):
    """Build the per-core Bass program (SPMD: same NEFF on every core)."""
    IC, OC, JC, KC, TS = I // 128, O // 128, J // 128, K // 128, TC // 128
    NTC, NKG, NEC = T // TC, K // KG, J // EC

    nc = bacc.Bacc("TRN2", target_bir_lowering=False, debug=False,
                   num_devices=num_devices)
    y = nc.dram_tensor("y", [T, I], F32, kind="ExternalInput").ap()
    Wy_w = nc.dram_tensor("Wy_w", [O, I], F32, kind="ExternalInput").ap()
    Wy_b = nc.dram_tensor("Wy_b", [O], F32, kind="ExternalInput").ap()
    Wz_w = nc.dram_tensor("Wz_w", [O, J], F32, kind="ExternalInput").ap()
    Wz_b = nc.dram_tensor("Wz_b", [O], F32, kind="ExternalInput").ap()
    dic_z = nc.dram_tensor("dic_z", [K, J], F32, kind="ExternalInput").ap()
    logp_in = nc.dram_tensor("logp_in", [K], F32, kind="ExternalInput").ap()
    invp_in = nc.dram_tensor("invp_in", [K], F32, kind="ExternalInput").ap()
    z = nc.dram_tensor("z", [T, J], F32, kind="ExternalOutput").ap()

    with tile.TileContext(nc) as tc, ExitStack() as stack:
        drp = stack.enter_context(tc.tile_pool(name="dram", bufs=1, space="DRAM"))
        y_d = drp.tile([T, I], dt_mm)
        wy_d = drp.tile([O, I], dt_mm)
        wz_d = drp.tile([O, J], dt_mm)
        dic_d = drp.tile([K, J], dt_mm)
        dzT_d = drp.tile([O, K], dt_mm)
        sums_d = drp.tile([T], F32)

        const = stack.enter_context(tc.tile_pool(name="const", bufs=1))
        logp = const.tile([128, KC], F32)
        nc.sync.dma_start(logp[:], logp_in.rearrange("(c p) -> p c", p=128))
        invp_f = const.tile([128, KC], F32)
        nc.sync.dma_start(invp_f[:], invp_in.rearrange("(c p) -> p c", p=128))
        invp = const.tile([128, KC], dt_mm)
        nc.vector.tensor_copy(invp[:], invp_f[:])
        wyb = const.tile([128, OC], F32)
        nc.sync.dma_start(wyb[:], Wy_b.rearrange("(c p) -> p c", p=128))
        wzb = const.tile([128, OC], F32)
        nc.sync.dma_start(wzb[:], Wz_b.rearrange("(c p) -> p c", p=128))
        wyT = const.tile([128, IC, O], dt_mm)

        hT_all = const.tile([128, OC, T], dt_mm)  # resident h.T for all chunks

        mps = stack.enter_context(tc.tile_pool(name="mps", bufs=3, space="PSUM"))
        spsp = stack.enter_context(tc.tile_pool(name="spsp", bufs=2, space="PSUM"))

        with tc.tile_pool(name="cast", bufs=8) as cast, \
             tc.tile_pool(name="wzt", bufs=1) as wztp, \
             tc.tile_pool(name="dzw", bufs=3) as dzw, \
             tc.tile_pool(name="stg", bufs=2) as stg:
            wzT = wztp.tile([128, JC, O], dt_mm)

            def cast_rows(src, dst, r0, r1):
                # f32 load (HWDGE) -> DVE cast -> bf16 store (HWDGE).
                # NB: the single SWDGE queue is ~125GB/s (and casts ~52GB/s),
                # so none of this may ride nc.gpsimd.
                for r in range(r0, r1):
                    cols = src.shape[1]
                    cf = cast.tile([128, max(I, J)], F32, tag="cf",
                                   name="cf", bufs=3)
                    nc.scalar.dma_start(cf[:, :cols], src[ts(r, 128), :])
                    ct = cast.tile([128, max(I, J)], dt_mm, tag="ct",
                                   name="ct", bufs=3)
                    nc.vector.tensor_copy(ct[:, :cols], cf[:, :cols])
                    nc.scalar.dma_start(dst[ts(r, 128), :], ct[:, :cols])

            def stage_h(tci):
                cast_rows(y, y_d, tci * (TC // 128), (tci + 1) * (TC // 128))

            def stage_d(kg):
                cast_rows(dic_z, dic_d, kg * (KG // 128), (kg + 1) * (KG // 128))

            def unit_h(tci):
                # yT transposes + hT matmuls for one token chunk
                yT = stg.tile([128, IC, TC], dt_mm, tag="yT")
                for ic in range(IC):
                    nc.sync.dma_start(yT[:, ic, :],
                                      y_d[ts(tci, TC), ts(ic, 128)],
                                      transpose=True)
                for oc in range(OC):
                    ps = mps.tile([128, TC], F32, tag="mm", name="ps")
                    for ic in range(IC):
                        nc.tensor.matmul(ps[:], wyT[:, ic, ts(oc, 128)],
                                         yT[:, ic, :],
                                         start=(ic == 0), stop=(ic == IC - 1))
                    nc.vector.tensor_scalar_add(hT_all[:, oc, ts(tci, TC)],
                                                ps[:], wyb[:, oc:oc + 1])

            def unit_d(kg):
                # dicT transposes + dz matmuls for one dictionary group
                dicT = dzw.tile([128, JC, KG], dt_mm, tag="dicT")
                for jc in range(JC):
                    nc.sync.dma_start(dicT[:, jc, :],
                                      dic_d[ts(kg, KG), ts(jc, 128)],
                                      transpose=True)
                for oc in range(OC):
                    ps = mps.tile([128, KG], F32, tag="mm", name="ps")
                    for jc in range(JC):
                        nc.tensor.matmul(ps[:], wzT[:, jc, ts(oc, 128)],
                                         dicT[:, jc, :],
                                         start=(jc == 0), stop=(jc == JC - 1))
                    so = dzw.tile([128, KG], dt_mm, tag="dzso")
                    nc.vector.tensor_scalar_add(so[:], ps[:],
                                                wzb[:, oc:oc + 1])
                    nc.scalar.dma_start(dzT_d[ts(oc, 128), ts(kg, KG)], so[:])

            # interleave hT chunks with dz groups; stage casts one unit ahead
            plan = []
            for i in range(max(NTC, NKG)):
                if i < NTC:
                    plan.append(("h", i))
                if i < NKG:
                    plan.append(("d", i))
            cast_rows(Wy_w, wy_d, 0, O // 128)
            cast_rows(Wz_w, wz_d, 0, O // 128)
            stage_h(0)
            for ic in range(IC):
                nc.sync.dma_start(wyT[:, ic, :], wy_d[:, ts(ic, 128)],
                                  transpose=True)
            for jc in range(JC):
                nc.sync.dma_start(wzT[:, jc, :], wz_d[:, ts(jc, 128)],
                                  transpose=True)
            for i, (kind, idx) in enumerate(plan):
                if i + 1 < len(plan):
                    k2, i2 = plan[i + 1]
                    (stage_h if k2 == "h" else stage_d)(i2)
                (unit_h if kind == "h" else unit_d)(idx)

        # ---- main per-token-chunk pipeline (logits/exp + weighted sum)
        mp = stack.enter_context(tc.tile_pool(name="mp", bufs=2))
        epp = stack.enter_context(tc.tile_pool(name="epp", bufs=1))
        zp = stack.enter_context(tc.tile_pool(name="zp", bufs=3))

        for tci in range(NTC):
            # Ep[p, kc, t] = exp(logits[kc*128+p, t]*scale + log prior)
            # sums[t] = sum_k exp(...) accumulated as (1/prior) row @ Ep
            Ep = epp.tile([128, KC, TC], dt_mm, tag="Ep")
            sps = spsp.tile([1, TC], F32, tag="sps", name="sps")
            for kc in range(KC):
                dzTk = mp.tile([128, OC, 128], dt_mm, tag="dzTk", bufs=6)
                nc.scalar.dma_start(
                    dzTk[:],
                    dzT_d[:, ts(kc, 128)].rearrange("(c p) m -> p c m", p=128))
                ps = mps.tile([128, TC], F32, tag="mm", name="ps")
                for oc in range(OC):
                    nc.tensor.matmul(ps[:], dzTk[:, oc, :],
                                     hT_all[:, oc, ts(tci, TC)],
                                     start=(oc == 0), stop=(oc == OC - 1))
                nc.scalar.activation(Ep[:, kc, :], ps[:], AF.Exp,
                                     bias=logp[:, kc:kc + 1], scale=scale)
                nc.tensor.matmul(sps[:], invp[:, kc:kc + 1], Ep[:, kc, :],
                                 start=(kc == 0), stop=(kc == KC - 1))
            # 1/sums, bounced through DRAM to spread over partitions
            srow = mp.tile([1, TC], F32, tag="srow")
            nc.vector.reciprocal(srow[:], sps[:])
            nc.gpsimd.dma_start(sums_d[ts(tci, TC)], srow[0:1, :])
            rsum = mp.tile([128, TS], F32, tag="rsum")
            nc.gpsimd.dma_start(
                rsum[:],
                sums_d[ts(tci, TC)].rearrange("(c p) -> p c", p=128))
            # weighted sum over the dictionary
            for ec in range(NEC):
                dicE = mp.tile([128, KC, EC], dt_mm, tag="dicE")
                nc.gpsimd.dma_start(
                    dicE[:],
                    dic_d[:, ts(ec, EC)].rearrange("(c p) e -> p c e", p=128))
                for tsi in range(TS):
                    zps = mps.tile([128, EC], F32, tag="zps", name="zps")
                    for kc in range(KC):
                        nc.tensor.matmul(zps[:], Ep[:, kc, ts(tsi, 128)],
                                         dicE[:, kc, :],
                                         start=(kc == 0), stop=(kc == KC - 1))
                    zt = zp.tile([128, EC], F32, tag="zt", name="zt")
                    nc.vector.tensor_scalar_mul(zt[:], zps[:],
                                                rsum[:, tsi:tsi + 1])
                    row0 = tci * TC + tsi * 128
                    nc.gpsimd.dma_start(z[row0:row0 + 128, ts(ec, EC)], zt[:])

    nc.compile()
    return nc


_NC_CACHE = {}


def _get_nc():
    key = "full"
    if key not in _NC_CACHE:
        _NC_CACHE[key] = build_bass(**FULL)
    return _NC_CACHE[key]


def make_in_maps(y, Wy_w, Wy_b, Wz_w, Wz_b, dic_z, prior):
    Bs = B // NCORES
    prior = np.asarray(prior, np.float32)
    shared = {
        "Wy_w": np.ascontiguousarray(np.asarray(Wy_w, np.float32)),
        "Wy_b": np.ascontiguousarray(np.asarray(Wy_b, np.float32)),
        "Wz_w": np.ascontiguousarray(np.asarray(Wz_w, np.float32)),
        "Wz_b": np.ascontiguousarray(np.asarray(Wz_b, np.float32)),
        "dic_z": np.ascontiguousarray(np.asarray(dic_z, np.float32)),
        "logp_in": np.log(prior).astype(np.float32),
        "invp_in": (1.0 / prior).astype(np.float32),
    }
    y = np.asarray(y, np.float32)
    return [{**shared,
             "y": np.ascontiguousarray(y[i * Bs:(i + 1) * Bs].reshape(Bs * N, EMB))}
            for i in range(NCORES)]


def run_spmd(in_maps, **kw):
    nc = _get_nc()
    res = bass_utils.run_bass_kernel_spmd(nc, in_maps,
                                          core_ids=list(range(NCORES)), **kw)
    Bs = B // NCORES
    z = np.concatenate(
        [res.results[i]["z"].reshape(Bs, N, 2048) for i in range(NCORES)],
        axis=0)
    return z.astype(np.float32), res


def kernel(y, Wy_w, Wy_b, Wz_w, Wz_b, dic_z, prior):
    """Full-input / full-output entry point (shards over B internally)."""
    z, _ = run_spmd(make_in_maps(y, Wy_w, Wy_b, Wz_w, Wz_b, dic_z, prior))
    return z


# revision 13
# speedup vs baseline: 1.0295x; 1.0295x over previous
"""Trainium2 Bass kernel for the retrieval-KNN module:

    h   = y @ Wy_w.T + Wy_b                      # [B,N,1024]
    dz  = dic_z @ Wz_w.T + Wz_b                  # [K,1024]
    att = softmax(h @ dz.T / sqrt(1024))         # [B,N,K]
    z   = einsum('bnk,k,ke->bne', att, prior, dic_z)

Strategy: data-parallel over B across 8 NeuronCores (8 batches = 2048
tokens per core); dic_z / weights replicated. All matmuls in bf16 (PE
fp32 runs at 1/4 rate), fp32 PSUM accumulation, transposes done by the
DMA transpose xbar on bf16 staged copies.  softmax has no max-subtraction
(logits are O(+-5) for this distribution) and folds the prior in as an
exp() bias: Ep = exp(logits/32 + log(prior)); then
z = (Ep.T @ dic_z) / (Ep.T @ (1/prior)) per token.
"""

import sys

import numpy as np


def _ensure_paths():
    for p in ("/opt/trn_rl_repo",):
        if p not in sys.path:
            sys.path.append(p)


_ensure_paths()

from contextlib import ExitStack  # noqa: E402

import concourse.bacc as bacc  # noqa: E402
import concourse.mybir as mybir  # noqa: E402
import concourse.tile as tile  # noqa: E402
from concourse import bass_utils  # noqa: E402
from concourse.bass import ts  # noqa: E402

F32 = mybir.dt.float32
BF16 = mybir.dt.bfloat16
AF = mybir.ActivationFunctionType

NCORES = 8
# Full problem dims (hardcoded per spec nn_Causal_v_69054484185473)
B, N, EMB = 64, 256, 1024
FULL = dict(T=(B // NCORES) * N, I=1024, O=1024, J=2048, K=4096,
            TC=512, KG=512, EC=512)
SCALE = 1.0 / 32.0  # 1/sqrt(EMB)


def build_bass(T=2048, I=1024, O=1024, J=2048, K=4096, TC=512, KG=512,
               EC=512, dt_mm=BF16, scale=SCALE, num_devices=NCORES):
    """Build the per-core Bass program (SPMD: same NEFF on every core)."""
    IC, OC, JC, KC, TS = I // 128, O // 128, J // 128, K // 128, TC // 128
    NTC, NKG, NEC = T // TC, K // KG, J // EC

    nc = bacc.Bacc("TRN2", target_bir_lowering=False, debug=False,
                   num_devices=num_devices)
    y = nc.dram_tensor("y", [T, I], F32, kind="ExternalInput").ap()
    Wy_w = nc.dram_tensor("Wy_w", [O, I], F32, kind="ExternalInput").ap()
    Wy_b = nc.dram_tensor("Wy_b", [O], F32, kind="ExternalInput").ap()
    Wz_w = nc.dram_tensor("Wz_w", [O, J], F32, kind="ExternalInput").ap()
    Wz_b = nc.dram_tensor("Wz_b", [O], F32, kind="ExternalInput").ap()
    dic_z = nc.dram_tensor("dic_z", [K, J], F32, kind="ExternalInput").ap()
    logp_in = nc.dram_tensor("logp_in", [K], F32, kind="ExternalInput").ap()
    invp_in = nc.dram_tensor("invp_in", [K], F32, kind="ExternalInput").ap()
    z = nc.dram_tensor("z", [T, J], F32, kind="ExternalOutput").ap()

    with tile.TileContext(nc) as tc, ExitStack() as stack:
        drp = stack.enter_context(tc.tile_pool(name="dram", bufs=1, space="DRAM"))
        y_d = drp.tile([T, I], dt_mm)
        wy_d = drp.tile([O, I], dt_mm)
        wz_d = drp.tile([O, J], dt_mm)
        dic_d = drp.tile([K, J], dt_mm)
        dzT_d = drp.tile([O, K], dt_mm)
        sums_d = drp.tile([T], F32)

        const = stack.enter_context(tc.tile_pool(name="const", bufs=1))
        logp = const.tile([128, KC], F32)
        nc.sync.dma_start(logp[:], logp_in.rearrange("(c p) -> p c", p=128))
        invp_f = const.tile([128, KC], F32)
        nc.sync.dma_start(invp_f[:], invp_in.rearrange("(c p) -> p c", p=128))
        invp = const.tile([128, KC], dt_mm)
        nc.vector.tensor_copy(invp[:], invp_f[:])
        wyb = const.tile([128, OC], F32)
        nc.sync.dma_start(wyb[:], Wy_b.rearrange("(c p) -> p c", p=128))
        wzb = const.tile([128, OC], F32)
        nc.sync.dma_start(wzb[:], Wz_b.rearrange("(c p) -> p c", p=128))
        wyT = const.tile([128, IC, O], dt_mm)

        hT_all = const.tile([128, OC, T], dt_mm)  # resident h.T for all chunks

        mps = stack.enter_context(tc.tile_pool(name="mps", bufs=3, space="PSUM"))
        spsp = stack.enter_context(tc.tile_pool(name="spsp", bufs=2, space="PSUM"))

        with tc.tile_pool(name="cast", bufs=8) as cast, \
             tc.tile_pool(name="wzt", bufs=1) as wztp, \
             tc.tile_pool(name="dzw", bufs=3) as dzw, \
             tc.tile_pool(name="stg", bufs=2) as stg:
            wzT = wztp.tile([128, JC, O], dt_mm)

            def cast_rows(src, dst, r0, r1):
                # f32 load (HWDGE) -> DVE cast -> bf16 store (HWDGE).
                # NB: the single SWDGE queue is ~125GB/s (and casts ~52GB/s),
                # so none of this may ride nc.gpsimd.
                for r in range(r0, r1):
                    cols = src.shape[1]
                    cf = cast.tile([128, max(I, J)], F32, tag="cf",
                                   name="cf", bufs=3)
                    nc.scalar.dma_start(cf[:, :cols], src[ts(r, 128), :])
                    ct = cast.tile([128, max(I, J)], dt_mm, tag="ct",
                                   name="ct", bufs=3)
                    nc.vector.tensor_copy(ct[:, :cols], cf[:, :cols])
                    nc.sync.dma_start(dst[ts(r, 128), :], ct[:, :cols])

            def stage_h(tci):
                cast_rows(y, y_d, tci * (TC // 128), (tci + 1) * (TC // 128))

            def stage_d(kg):
                cast_rows(dic_z, dic_d, kg * (KG // 128), (kg + 1) * (KG // 128))

            def unit_h(tci):
                # yT transposes + hT matmuls for one token chunk
                yT = stg.tile([128, IC, TC], dt_mm, tag="yT")
                for ic in range(IC):
                    nc.sync.dma_start(yT[:, ic, :],
                                      y_d[ts(tci, TC), ts(ic, 128)],
                                      transpose=True)
                for oc in range(OC):
                    ps = mps.tile([128, TC], F32, tag="mm", name="ps")
                    for ic in range(IC):
                        nc.tensor.matmul(ps[:], wyT[:, ic, ts(oc, 128)],
                                         yT[:, ic, :],
                                         start=(ic == 0), stop=(ic == IC - 1))
                    nc.vector.tensor_scalar_add(hT_all[:, oc, ts(tci, TC)],
                                                ps[:], wyb[:, oc:oc + 1])

            def unit_d(kg):
                # dicT transposes + dz matmuls for one dictionary group
                dicT = dzw.tile([128, JC, KG], dt_mm, tag="dicT")
                for jc in range(JC):
                    nc.sync.dma_start(dicT[:, jc, :],
                                      dic_d[ts(kg, KG), ts(jc, 128)],
                                      transpose=True)
                for oc in range(OC):
                    ps = mps.tile([128, KG], F32, tag="mm", name="ps")
                    for jc in range(JC):
                        nc.tensor.matmul(ps[:], wzT[:, jc, ts(oc, 128)],
                                         dicT[:, jc, :],
                                         start=(jc == 0), stop=(jc == JC - 1))
                    so = dzw.tile([128, KG], dt_mm, tag="dzso")
                    nc.vector.tensor_scalar_add(so[:], ps[:],
                                                wzb[:, oc:oc + 1])
                    nc.gpsimd.dma_start(dzT_d[ts(oc, 128), ts(kg, KG)], so[:])

            # interleave hT chunks with dz groups; stage casts one unit ahead
            plan = []
            for i in range(max(NTC, NKG)):
                if i < NTC:
                    plan.append(("h", i))
                if i < NKG:
                    plan.append(("d", i))
            cast_rows(Wy_w, wy_d, 0, O // 128)
            cast_rows(Wz_w, wz_d, 0, O // 128)
            stage_h(0)
            for ic in range(IC):
                nc.sync.dma_start(wyT[:, ic, :], wy_d[:, ts(ic, 128)],
                                  transpose=True)
            for jc in range(JC):
                nc.sync.dma_start(wzT[:, jc, :], wz_d[:, ts(jc, 128)],
                                  transpose=True)
            for i, (kind, idx) in enumerate(plan):
                if i + 1 < len(plan):
                    k2, i2 = plan[i + 1]
                    (stage_h if k2 == "h" else stage_d)(i2)
                (unit_h if kind == "h" else unit_d)(idx)

        # ---- main per-token-chunk pipeline (logits/exp + weighted sum)
        mp = stack.enter_context(tc.tile_pool(name="mp", bufs=2))
        epp = stack.enter_context(tc.tile_pool(name="epp", bufs=1))
        zp = stack.enter_context(tc.tile_pool(name="zp", bufs=3))

        for tci in range(NTC):
            # Ep[p, kc, t] = exp(logits[kc*128+p, t]*scale + log prior)
            # sums[t] = sum_k exp(...) accumulated as (1/prior) row @ Ep
            Ep = epp.tile([128, KC, TC], dt_mm, tag="Ep")
            sps = spsp.tile([1, TC], F32, tag="sps", name="sps")
            for kc in range(KC):
                dzTk = mp.tile([128, OC, 128], dt_mm, tag="dzTk", bufs=6)
                nc.scalar.dma_start(
                    dzTk[:],
                    dzT_d[:, ts(kc, 128)].rearrange("(c p) m -> p c m", p=128))
                ps = mps.tile([128, TC], F32, tag="mm", name="ps")
                for oc in range(OC):
                    nc.tensor.matmul(ps[:], dzTk[:, oc, :],
                                     hT_all[:, oc, ts(tci, TC)],
                                     start=(oc == 0), stop=(oc == OC - 1))
                nc.scalar.activation(Ep[:, kc, :], ps[:], AF.Exp,
                                     bias=logp[:, kc:kc + 1], scale=scale)
                nc.tensor.matmul(sps[:], invp[:, kc:kc + 1], Ep[:, kc, :],
                                 start=(kc == 0), stop=(kc == KC - 1))
            # 1/sums, bounced through DRAM to spread over partitions
            srow = mp.tile([1, TC], F32, tag="srow")
            nc.vector.reciprocal(srow[:], sps[:])
            nc.gpsimd.dma_start(sums_d[ts(tci, TC)], srow[0:1, :])
            rsum = mp.tile([128, TS], F32, tag="rsum")
            nc.gpsimd.dma_start(
                rsum[:],
                sums_d[ts(tci, TC)].rearrange("(c p) -> p c", p=128))
            # weighted sum over the dictionary
            for ec in range(NEC):
                dicE = mp.tile([128, KC, EC], dt_mm, tag="dicE")
                nc.gpsimd.dma_start(
                    dicE[:],
                    dic_d[:, ts(ec, EC)].rearrange("(c p) e -> p c e", p=128))
                for tsi in range(TS):
                    zps = mps.tile([128, EC], F32, tag="zps", name="zps")
                    for kc in range(KC):
                        nc.tensor.matmul(zps[:], Ep[:, kc, ts(tsi, 128)],
                                         dicE[:, kc, :],
                                         start=(kc == 0), stop=(kc == KC - 1))
                    zt = zp.tile([128, EC], F32, tag="zt", name="zt")
                    nc.vector.tensor_scalar_mul(zt[:], zps[:],
                                                rsum[:, tsi:tsi + 1])
                    row0 = tci * TC + tsi * 128
                    nc.gpsimd.dma_start(z[row0:row0 + 128, ts(ec, EC)], zt[:])

    nc.compile()
    return nc


_NC_CACHE = {}


def _get_nc():
    key = "full"
    if key not in _NC_CACHE:
        _NC_CACHE[key] = build_bass(**FULL)
    return _NC_CACHE[key]


def make_in_maps(y, Wy_w, Wy_b, Wz_w, Wz_b, dic_z, prior):
    Bs = B // NCORES
    prior = np.asarray(prior, np.float32)
    shared = {
        "Wy_w": np.ascontiguousarray(np.asarray(Wy_w, np.float32)),
        "Wy_b": np.ascontiguousarray(np.asarray(Wy_b, np.float32)),
        "Wz_w": np.ascontiguousarray(np.asarray(Wz_w, np.float32)),
        "Wz_b": np.ascontiguousarray(np.asarray(Wz_b, np.float32)),
        "dic_z": np.ascontiguousarray(np.asarray(dic_z, np.float32)),
        "logp_in": np.log(prior).astype(np.float32),
        "invp_in": (1.0 / prior).astype(np.float32),
    }
    y = np.asarray(y, np.float32)
    return [{**shared,
             "y": np.ascontiguousarray(y[i * Bs:(i + 1) * Bs].reshape(Bs * N, EMB))}
            for i in range(NCORES)]


def run_spmd(in_maps, **kw):
    nc = _get_nc()
    res = bass_utils.run_bass_kernel_spmd(nc, in_maps,
                                          core_ids=list(range(NCORES)), **kw)
    Bs = B // NCORES
    z = np.concatenate(
        [res.results[i]["z"].reshape(Bs, N, 2048) for i in range(NCORES)],
        axis=0)
    return z.astype(np.float32), res


def kernel(y, Wy_w, Wy_b, Wz_w, Wz_b, dic_z, prior):
    """Full-input / full-output entry point (shards over B internally)."""
    z, _ = run_spmd(make_in_maps(y, Wy_w, Wy_b, Wz_w, Wz_b, dic_z, prior))
    return z
